# revision 1
# baseline (speedup 1.0000x reference)
"""Weighted 2D cross-entropy (BCE-over-classes) loss on 8 Trainium2 cores.

Math (matches the reference):
  t in [0,19); pos = t>0, neg = t==0 (all pixels are pos or neg; mask == 1)
  S(i) = sum_c bce(i,c) = -[ B(i) + A(i) ]
     A(i) = sum_c log(1-p_c(i))
     B(i) = log(p_t(i)) - log(1-p_t(i))
  loss = ( (NEG/TOT)*S_pos_sum + (POS/TOT)*S_neg_sum ) / (TOT*C)

Per-core (core k <- batch element k, pure data parallel):
  - one full-grid ACT pass: L_c = Ln(1-p_c) in bf16, accum_out -> U_all
  - one full-grid DVE pass: masked_c = (T==c)*L_c (fused scalar_tensor_tensor)
  - PE identity-matmuls accumulate A = sum_c L_c and L_sel = sum_c masked_c
    into PSUM (f32) -- the gather of log(1-p) at the target class.
  - per-pixel tail: B = log(1-exp(L_sel)) - L_sel on ACT; masked sums via
    accum_out; final partition reduce via ones-matmul -> 8 scalars per core.
Host combines the 8x8 scalars in float64 (the "all-reduce").
"""

from contextlib import ExitStack

import numpy as np

import concourse.bass as bass
import concourse.mybir as mybir
import concourse.tile as tile
from concourse import bacc
from concourse.bass_utils import run_bass_kernel_spmd

# problem shape (hardcoded per harness contract)
N, C, H, W = 8, 19, 512, 1024
PIX = H * W          # 524288 pixels per core
P = 128              # partitions
FCOLS = PIX // P     # 4096 free columns when pixels laid out [128, 4096]
FT = 1024            # pixel-tile free width
NTILES = FCOLS // FT # 4 pixel tiles per core
N_CORES = 8

DT = mybir.dt

# stats buffer column layout (all f32; per-tile partial sums, 4 cols per group)
#   sum A (=U_all), sum logp, sum pos*A, sum pos*logp, sum pos*L_sel,
#   sum L_sel, pos count
COL_UALL = 0
COL_LOGP = 4
COL_POSA = 8
COL_POSLOGP = 12
COL_POSLSEL = 16
COL_LSEL = 20
COL_CNT = 24
NSTAT = 7  # number of final scalars (one per group above)
STAT_COLS = 32


def build_kernel() -> bass.Bass:
    # Bacc (not raw Bass): its compile() pipeline runs
    # generate_event_semaphores, which splits multi-sem waits to satisfy the
    # 1-wait-per-instruction TRN2 sync structs -- raw Bass modules with
    # Tile-emitted multi-waits fail walrus codegen.
    nc = bacc.Bacc("TRN2")

    predict = nc.declare_dram_parameter("predict", [C, PIX], DT.float32, isOutput=False)
    target = nc.declare_dram_parameter("target", [P, FCOLS], DT.int32, isOutput=False)
    idn = nc.declare_dram_parameter("idn", [P, P], DT.bfloat16, isOutput=False)
    out = nc.declare_dram_parameter("out", [1, NSTAT], DT.float32, isOutput=True)

    pred_r = predict.rearrange("c (p f) -> c p f", p=P)  # [19, 128, 4096]

    with tile.TileContext(nc) as tc, ExitStack() as ctx:
        const = ctx.enter_context(tc.tile_pool(name="const", bufs=1))
        p_pool = ctx.enter_context(tc.tile_pool(name="p", bufs=8))
        lm_pool = ctx.enter_context(tc.tile_pool(name="lm", bufs=21))
        pix_pool = ctx.enter_context(tc.tile_pool(name="pix", bufs=2))
        scr_pool = ctx.enter_context(tc.tile_pool(name="scr", bufs=2))
        eq_pool = ctx.enter_context(tc.tile_pool(name="eq", bufs=4))
        psum_pool = ctx.enter_context(tc.tile_pool(name="ps", bufs=2, space="PSUM"))

        idn_sb = const.tile([P, P], DT.bfloat16, tag="idn")
        nc.sync.dma_start(out=idn_sb[:], in_=idn[:])

        t_i32 = const.tile([P, FCOLS], DT.int32, tag="ti")
        nc.sync.dma_start(out=t_i32[:], in_=target[:])
        t_bf = const.tile([P, FCOLS], DT.bfloat16, tag="tb")
        nc.vector.tensor_copy(out=t_bf[:], in_=t_i32[:])

        stats = const.tile([P, STAT_COLS], DT.float32, tag="stats")
        nc.vector.memset(stats[:], 0.0)

        # pos counts up-front (also settles the DVE self-dep on t_bf so later
        # scalar_tensor_tensor ops carry at most one sem wait -- the STT
        # hardware sync struct only holds a single wait condition)
        cnt_scr = const.tile([P, FT], DT.bfloat16, tag="cntscr")
        for t in range(NTILES):
            nc.vector.tensor_scalar(
                out=cnt_scr[:],
                in0=t_bf[:, t * FT : (t + 1) * FT],
                scalar1=0.5,
                scalar2=None,
                op0=mybir.AluOpType.is_gt,
                op1=mybir.AluOpType.add,
                accum_out=stats[:, COL_CNT + t : COL_CNT + t + 1],
            )

        for t in range(NTILES):
            fsl = slice(t * FT, (t + 1) * FT)
            t_sl = t_bf[:, fsl]

            # PSUM accumulator: [:, :FT] = A, [:, FT:] = L_sel   (4 banks)
            acc_ps = psum_pool.tile([P, 2 * FT], DT.float32, tag="acc")

            for c in range(C):
                p_t = p_pool.tile([P, FT], DT.float32, tag="p")
                # p bufs=8 aligns slot reuse with the global DMA->DMAHW-proc
                # round-robin (8 procs), so the WAW on the old writer is
                # same-proc FIFO order and Tile emits no cross-queue wait
                nc.sync.dma_start(out=p_t[:], in_=pred_r[c, :, fsl])

                # lm[:, :FT] = L_c = Ln(1-p) bf16 ; lm[:, FT:] = (T==c)*L_c
                lm = lm_pool.tile([P, 2 * FT], DT.bfloat16, tag="lm")
                nc.scalar.activation(
                    out=lm[:, :FT],
                    in_=p_t[:],
                    func=mybir.ActivationFunctionType.Ln,
                    bias=1.0,
                    scale=-1.0,
                )
                # eq at DVE 4x (16-bit tensor_scalar) + mult at 2x beats the
                # fused scalar_tensor_tensor, which only has a 1x uop
                eq = eq_pool.tile([P, FT], DT.bfloat16, tag="eq")
                nc.vector.tensor_scalar(
                    out=eq[:],
                    in0=t_sl,
                    scalar1=float(c),
                    scalar2=None,
                    op0=mybir.AluOpType.is_equal,
                )
                nc.vector.tensor_mul(out=lm[:, FT:], in0=eq[:], in1=lm[:, :FT])

                for s in range(4):
                    ssl = slice(s * 512, (s + 1) * 512)
                    nc.tensor.matmul(
                        acc_ps[:, ssl],
                        lhsT=idn_sb[:],
                        rhs=lm[:, ssl],
                        start=(c == 0),
                        stop=(c == C - 1),
                    )

            a_ps = acc_ps[:, :FT]
            lsel_ps = acc_ps[:, FT:]

            # expL = exp(L_sel) = 1-p_t ;  logp = Ln(1 - expL) = log(p_t)
            expl = pix_pool.tile([P, FT], DT.float32, tag="expl")
            nc.scalar.activation(
                out=expl[:], in_=lsel_ps, func=mybir.ActivationFunctionType.Exp
            )
            logp = pix_pool.tile([P, FT], DT.float32, tag="logp")
            nc.scalar.activation(
                out=logp[:],
                in_=expl[:],
                func=mybir.ActivationFunctionType.Ln,
                bias=1.0,
                scale=-1.0,
            )
            # sum A  (= U_all contribution)
            nc.vector.tensor_reduce(
                out=stats[:, COL_UALL + t : COL_UALL + t + 1],
                in_=a_ps,
                axis=mybir.AxisListType.X,
                op=mybir.AluOpType.add,
            )
            # sum logp
            nc.vector.tensor_reduce(
                out=stats[:, COL_LOGP + t : COL_LOGP + t + 1],
                in_=logp[:],
                axis=mybir.AxisListType.X,
                op=mybir.AluOpType.add,
            )

            scr = scr_pool.tile([P, FT], DT.float32, tag="scr")
            # sum pos*A
            nc.vector.scalar_tensor_tensor(
                out=scr[:],
                in0=t_sl,
                scalar=0.5,
                in1=a_ps,
                op0=mybir.AluOpType.is_gt,
                op1=mybir.AluOpType.mult,
                accum_out=stats[:, COL_POSA + t : COL_POSA + t + 1],
            )
            # sum pos*logp
            nc.vector.scalar_tensor_tensor(
                out=scr[:],
                in0=t_sl,
                scalar=0.5,
                in1=logp[:],
                op0=mybir.AluOpType.is_gt,
                op1=mybir.AluOpType.mult,
                accum_out=stats[:, COL_POSLOGP + t : COL_POSLOGP + t + 1],
            )
            # sum pos*L_sel
            nc.vector.scalar_tensor_tensor(
                out=scr[:],
                in0=t_sl,
                scalar=0.5,
                in1=lsel_ps,
                op0=mybir.AluOpType.is_gt,
                op1=mybir.AluOpType.mult,
                accum_out=stats[:, COL_POSLSEL + t : COL_POSLSEL + t + 1],
            )
            # sum L_sel
            nc.vector.tensor_reduce(
                out=stats[:, COL_LSEL + t : COL_LSEL + t + 1],
                in_=lsel_ps,
                axis=mybir.AxisListType.X,
                op=mybir.AluOpType.add,
            )

        # fold each stat group into one column, then partition-reduce via matmul
        finals = const.tile([P, NSTAT], DT.float32, tag="finals")
        groups = [
            (COL_UALL, NTILES),
            (COL_LOGP, NTILES),
            (COL_POSA, NTILES),
            (COL_POSLOGP, NTILES),
            (COL_POSLSEL, NTILES),
            (COL_LSEL, NTILES),
            (COL_CNT, NTILES),
        ]
        for g, (start, width) in enumerate(groups):
            nc.vector.tensor_reduce(
                out=finals[:, g : g + 1],
                in_=stats[:, start : start + width],
                axis=mybir.AxisListType.X,
                op=mybir.AluOpType.add,
            )

        out_sb = const.tile([1, NSTAT], DT.float32, tag="outsb")
        nc.gpsimd.tensor_reduce(
            out=out_sb[:],
            in_=finals[:],
            axis=mybir.AxisListType.C,
            op=mybir.AluOpType.add,
        )
        nc.gpsimd.dma_start(out=out[:], in_=out_sb[:])

    if not nc.is_finalized():
        nc.finalize()

    return nc
    for f in nc.m.functions:
        for bb in f.blocks:
            il = bb.instructions
            i = 0
            n_split = 0
            while i < len(il):
                ins = il[i]
                i += 1
                if ins.opcode == "Drain" or ins.sync_info is None:
                    continue
                w = ins.sync_info.on_wait
                if not w or len(w) < 2:
                    continue
                if ins.opcode == "DMACopy" and len(w) == 2:
                    act = [x for x in w if x.ant_name.startswith("Activation")]
                    hw = [x for x in w if x.ant_name.startswith("DMAHW")]
                    if len(act) == 1 and len(hw) == 1:
                        ins.sync_info = mybir.SyncInfo(
                            on_wait=act, on_update=ins.sync_info.on_update
                        )
                        continue
                for j, extra in enumerate(list(w)[:-1]):
                    drain = mybir.InstDrain(
                        name=f"{ins.name}-waitsplit{j}",
                        engine=ins.engine,
                        sync_info=mybir.SyncInfo(on_wait=[extra], on_update=[]),
                    )
                    il.insert(i - 1, drain)
                    i += 1
                ins.sync_info = mybir.SyncInfo(
                    on_wait=[w[-1]], on_update=ins.sync_info.on_update
                )
                n_split += 1

    return nc


_NC_CACHE = None


def kernel(predict: np.ndarray, target: np.ndarray) -> np.ndarray:
    global _NC_CACHE
    if _NC_CACHE is None:
        _NC_CACHE = build_kernel()
    nc = _NC_CACHE

    import ml_dtypes

    predict = np.ascontiguousarray(predict, dtype=np.float32)
    target = np.ascontiguousarray(target, dtype=np.int32)
    idn = np.eye(P, dtype=np.float32).astype(ml_dtypes.bfloat16)

    in_maps = []
    for k in range(N_CORES):
        in_maps.append(
            {
                "predict": predict[k].reshape(C, PIX),
                "target": target[k].reshape(P, FCOLS),
                "idn": idn,
            }
        )

    res = run_bass_kernel_spmd(nc, in_maps, list(range(N_CORES)))

    tot = np.float64(0.0)
    s_all = np.float64(0.0)
    s_pos = np.float64(0.0)
    pos = np.float64(0.0)
    for k in range(N_CORES):
        st = res.results[k]["out"].reshape(-1).astype(np.float64)
        u_all, logp_s, pos_a, pos_logp, pos_lsel, lsel_s, cnt = st[:NSTAT]
        v_all = logp_s - lsel_s
        v_pos = pos_logp - pos_lsel
        s_all += -(v_all + u_all)
        s_pos += -(v_pos + pos_a)
        pos += cnt
        tot += PIX
    neg = tot - pos
    s_neg = s_all - s_pos
    loss = ((neg / tot) * s_pos + (pos / tot) * s_neg) / (tot * C)
    return np.float32(loss)



# revision 7
# speedup vs baseline: 1.0650x; 1.0650x over previous
"""Weighted 2D cross-entropy (BCE-over-classes) loss on 8 Trainium2 cores.

Math (matches the reference):
  t in [0,19); pos = t>0, neg = t==0 (all pixels are pos or neg; mask == 1)
  A(i)    = sum_c log(1-p_c(i))
  lsel(i) = log(1-p_t(i))     (gather at the target class)
  logp(i) = log(p_t(i)) = log(1-exp(lsel(i)))
  S(i)    = sum_c bce(i,c) = -(A(i) + logp(i) - lsel(i))
  loss = ( (NEG/TOT)*S_pos_sum + (POS/TOT)*S_neg_sum ) / (TOT*C)

Per-core (core k <- batch element k, pure data parallel), per pixel-tile:
  - ACT: L_c = Ln(1-p_c) bf16, accum_out -> per-class columns of U=sum A (free)
  - DVE: eq_c = (T==c), masked_c = eq_c*L_c; class 0 uses a fused
    tensor_tensor_reduce whose accum_out is sum(eq0*L_0) = sum_neg lsel (free)
  - PE : identity matmuls accumulate A and L_sel into PSUM f32
  - tail: exp/ln on ACT give logp (ln's accum_out -> sum logp);
    DVE TTRs against the eq0 mask give sum_neg A and sum_neg logp;
    one tensor_reduce gives sum lsel.
  - stats land in two SBUF tiles (one ACT-written, one DVE-written) and are
    DMA'd out as [128, 96]; host does the partition reduce + all-reduce in f64.
POS/NEG counts are exact int counts taken from the target array on the host
(the "all-reduce" of the sharding hint).

Perf notes vs v1 baseline (145us):
  - PE work halved (no pos-masked second accumulation); PE stays HAM-warm.
  - ACT Ln+Exp forced into the one table set that holds both -> no 2.6us
    table thrash per pixel-tile.
  - target shipped as uint8 (0.5MB vs 2MB) on the scalar HWDGE queue so the
    sync queue streams predict from instruction 0.
  - final partition reduce moved to the host (49KB DMA) - kills the gpsimd
    cross-lane reduce + drain from the tail.
"""

from contextlib import ExitStack

import numpy as np

import concourse.bass as bass
import concourse.mybir as mybir
import concourse.tile as tile
from concourse import bacc
from concourse.bass_utils import run_bass_kernel_spmd

# problem shape (hardcoded per harness contract)
N, C, H, W = 8, 19, 512, 1024
PIX = H * W          # 524288 pixels per core
P = 128              # partitions
FCOLS = PIX // P     # 4096 free columns when pixels laid out [128, 4096]
FT = 1024            # pixel-tile free width
NTILES = FCOLS // FT # 4 pixel tiles per core
N_CORES = 8

DT = mybir.dt

# stats column layout
# ACT-written tile: cols [t*C + c] = per-(tile,class) colsum of Ln(1-p)  (76)
#                   cols [C*NTILES + t] = per-tile colsum of logp        (4)
ACT_COLS = C * NTILES + NTILES  # 80
# DVE-written tile: [0:4]=neg_a, [4:8]=neg_logp, [8:12]=lsel_sum, [12:16]=neg_lsel
# (+ [16:20]=u_all, [20:24]=logp_sum when ACT accum_out is disabled)
DVE_COLS = 6 * NTILES  # 24
OUT_COLS = ACT_COLS + DVE_COLS  # 104

# feature flags (bisection / fallbacks)
USE_ACT_ACCUM = True   # ACT activation accum_out for u_all / logp sums
PATCH_TABLES = True    # force Ln+Exp into one ACT table set
USE_SCALAR_DMA = True  # small DMAs on the scalar engine HWDGE ring
USE_U8 = True          # ship target as uint8 instead of int32


def _patch_act_tables():
    """Force Ln+Exp onto the single table set that contains both.

    bacc's insert_act_table_loads greedily maps each activation function to
    the first table set containing it (Ln -> natural_log, Exp ->
    exp_and_others), which makes every per-tile exp/ln tail pay two 1.28us
    ACT_TABLE_LOADs.  One real set (natural_log_exp_and_others) holds both
    functions; masking Ln/Exp out of every other set's advertised membership
    (names/order unchanged, so set ids stay valid) makes the pass emit a
    single load at kernel entry.
    """
    import concourse.bacc as bacc_mod

    if getattr(bacc_mod, "_act_tbl_patched", False):
        return
    fLn = mybir.ActivationFunctionType.Ln
    fExp = mybir.ActivationFunctionType.Exp
    orig = bacc_mod.get_activation_tables

    def patched(arch):
        tbl = orig(arch)
        combined = [n for n, fs in tbl.items() if fLn in fs and fExp in fs]
        if not combined:
            return tbl
        keep = combined[0]
        return {
            n: (set(fs) if n == keep else set(fs) - {fLn, fExp})
            for n, fs in tbl.items()
        }

    bacc_mod.get_activation_tables = patched
    bacc_mod._act_tbl_patched = True


def build_kernel() -> bass.Bass:
    if PATCH_TABLES:
        _patch_act_tables()

    # Bacc (not raw Bass): its compile() pipeline runs
    # generate_event_semaphores, which splits multi-sem waits to satisfy the
    # 1-wait-per-instruction TRN2 sync structs.
    nc = bacc.Bacc("TRN2")

    predict = nc.declare_dram_parameter("predict", [C, PIX], DT.float32, isOutput=False)
    tdt = DT.uint8 if USE_U8 else DT.int32
    target = nc.declare_dram_parameter("target", [P, FCOLS], tdt, isOutput=False)
    idn = nc.declare_dram_parameter("idn", [P, P], DT.bfloat16, isOutput=False)
    out = nc.declare_dram_parameter("out", [P, OUT_COLS], DT.float32, isOutput=True)

    pred_r = predict.rearrange("c (p f) -> c p f", p=P)  # [19, 128, 4096]

    with tile.TileContext(nc) as tc, ExitStack() as ctx:
        const = ctx.enter_context(tc.tile_pool(name="const", bufs=1))
        p_pool = ctx.enter_context(tc.tile_pool(name="p", bufs=8))
        l_pool = ctx.enter_context(tc.tile_pool(name="l", bufs=10))
        mk_pool = ctx.enter_context(tc.tile_pool(name="mk", bufs=6))
        eq_pool = ctx.enter_context(tc.tile_pool(name="eq", bufs=4))
        tb_pool = ctx.enter_context(tc.tile_pool(name="tb", bufs=2))
        pix_pool = ctx.enter_context(tc.tile_pool(name="pix", bufs=4))
        scr_pool = ctx.enter_context(tc.tile_pool(name="scr", bufs=2))
        psum_pool = ctx.enter_context(tc.tile_pool(name="ps", bufs=2, space="PSUM"))

        # small loads go on the scalar engine's HWDGE ring (qActDynamicHW) so
        # the sync ring (qSPDynamicHW) carries nothing but the predict stream
        small_dma = nc.scalar if USE_SCALAR_DMA else nc.sync
        idn_sb = const.tile([P, P], DT.bfloat16, tag="idn")
        small_dma.dma_start(out=idn_sb[:], in_=idn[:])
        t_u8 = const.tile([P, FCOLS], tdt, tag="tu8")
        small_dma.dma_start(out=t_u8[:], in_=target[:])

        stats = const.tile([P, OUT_COLS], DT.float32, tag="stats")
        AC = ACT_COLS  # DVE columns live at stats[:, AC + i]
        nc.vector.memset(stats[:], 0.0)

        for t in range(NTILES):
            fsl = slice(t * FT, (t + 1) * FT)

            t_bf = tb_pool.tile([P, FT], DT.bfloat16, tag="tb")
            nc.vector.tensor_copy(out=t_bf[:], in_=t_u8[:, fsl])

            # PSUM accumulator: [:, :FT] = A, [:, FT:] = L_sel   (4 banks)
            acc = psum_pool.tile([P, 2 * FT], DT.float32, tag="acc")

            for c in range(C):
                p_t = p_pool.tile([P, FT], DT.float32, tag="p")
                # p bufs=8 aligns slot reuse with the 8 DMAHW sem lanes, so
                # the WAW on the old writer is same-lane FIFO order
                nc.sync.dma_start(out=p_t[:], in_=pred_r[c, :, fsl])

                L = l_pool.tile([P, FT], DT.bfloat16, tag="l")
                nc.scalar.activation(
                    out=L[:],
                    in_=p_t[:],
                    func=mybir.ActivationFunctionType.Ln,
                    bias=1.0,
                    scale=-1.0,
                    **(
                        dict(accum_out=stats[:, t * C + c : t * C + c + 1])
                        if USE_ACT_ACCUM
                        else {}
                    ),
                )

                mk = mk_pool.tile([P, FT], DT.bfloat16, tag="mk")
                # eq at DVE 4x (16-bit tensor_scalar) + mult at 2x
                eqc = eq_pool.tile([P, FT], DT.bfloat16, tag="eq")
                nc.vector.tensor_scalar(
                    out=eqc[:],
                    in0=t_bf[:],
                    scalar1=float(c),
                    scalar2=None,
                    op0=mybir.AluOpType.is_equal,
                )
                nc.vector.tensor_mul(out=mk[:], in0=eqc[:], in1=L[:])
                if c == 0:
                    # masked_0 is nonzero exactly on neg pixels (t==0 selects
                    # class 0), so its colsum is sum over neg pixels of lsel
                    nc.vector.tensor_reduce(
                        out=stats[:, AC + 3 * NTILES + t : AC + 3 * NTILES + t + 1],
                        in_=mk[:],
                        axis=mybir.AxisListType.X,
                        op=mybir.AluOpType.add,
                    )

                for s in range(2):
                    ssl = slice(s * 512, (s + 1) * 512)
                    nc.tensor.matmul(
                        acc[:, ssl],
                        lhsT=idn_sb[:],
                        rhs=L[:, ssl],
                        start=(c == 0),
                        stop=(c == C - 1),
                    )
                for s in range(2):
                    nc.tensor.matmul(
                        acc[:, FT + s * 512 : FT + (s + 1) * 512],
                        lhsT=idn_sb[:],
                        rhs=mk[:, s * 512 : (s + 1) * 512],
                        start=(c == 0),
                        stop=(c == C - 1),
                    )

            a_ps = acc[:, :FT]
            lsel_ps = acc[:, FT:]

            # expl = exp(lsel) = 1-p_t ;  logp = Ln(1-expl) = log(p_t)
            expl = pix_pool.tile([P, FT], DT.float32, tag="expl")
            nc.scalar.activation(
                out=expl[:], in_=lsel_ps, func=mybir.ActivationFunctionType.Exp
            )
            logp = pix_pool.tile([P, FT], DT.float32, tag="logp")
            nc.scalar.activation(
                out=logp[:],
                in_=expl[:],
                func=mybir.ActivationFunctionType.Ln,
                bias=1.0,
                scale=-1.0,
                **(
                    dict(accum_out=stats[:, C * NTILES + t : C * NTILES + t + 1])
                    if USE_ACT_ACCUM
                    else {}
                ),
            )

            # sum lsel
            nc.vector.tensor_reduce(
                out=stats[:, AC + 2 * NTILES + t : AC + 2 * NTILES + t + 1],
                in_=lsel_ps,
                axis=mybir.AxisListType.X,
                op=mybir.AluOpType.add,
            )
            scr = scr_pool.tile([P, FT], DT.float32, tag="scr")
            # sum over neg pixels of A   ((t<0.5)*A summed, as in v1's STTs)
            nc.vector.scalar_tensor_tensor(
                out=scr[:],
                in0=t_bf[:],
                scalar=0.5,
                in1=a_ps,
                op0=mybir.AluOpType.is_lt,
                op1=mybir.AluOpType.mult,
                accum_out=stats[:, AC + t : AC + t + 1],
            )
            # sum over neg pixels of logp
            nc.vector.scalar_tensor_tensor(
                out=scr[:],
                in0=t_bf[:],
                scalar=0.5,
                in1=logp[:],
                op0=mybir.AluOpType.is_lt,
                op1=mybir.AluOpType.mult,
                accum_out=stats[:, AC + NTILES + t : AC + NTILES + t + 1],
            )
            if not USE_ACT_ACCUM:
                # DVE fallbacks: u_all = colsum(A_psum), logp_s = colsum(logp)
                nc.vector.tensor_reduce(
                    out=stats[:, AC + 4 * NTILES + t : AC + 4 * NTILES + t + 1],
                    in_=a_ps,
                    axis=mybir.AxisListType.X,
                    op=mybir.AluOpType.add,
                )
                nc.vector.tensor_reduce(
                    out=stats[:, AC + 5 * NTILES + t : AC + 5 * NTILES + t + 1],
                    in_=logp[:],
                    axis=mybir.AxisListType.X,
                    op=mybir.AluOpType.add,
                )

        small_dma.dma_start(out=out[:], in_=stats[:])

    if not nc.is_finalized():
        nc.finalize()

    return nc


def make_in_maps(predict: np.ndarray, target: np.ndarray) -> list:
    import ml_dtypes

    predict = np.ascontiguousarray(predict, dtype=np.float32)
    idn = np.eye(P, dtype=np.float32).astype(ml_dtypes.bfloat16)
    in_maps = []
    for k in range(N_CORES):
        in_maps.append(
            {
                "predict": predict[k].reshape(C, PIX),
                "target": np.ascontiguousarray(
                    target[k]
                    .reshape(P, FCOLS)
                    .astype(np.uint8 if USE_U8 else np.int32)
                ),
                "idn": idn,
            }
        )
    return in_maps


def combine_results(results: list, target: np.ndarray) -> np.float32:
    """Host-side partition reduce + all-reduce + final scalar combine."""
    tot = np.float64(N * PIX)
    neg = np.float64(np.count_nonzero(np.asarray(target) == 0))
    pos = tot - neg

    s_all = np.float64(0.0)
    s_neg = np.float64(0.0)
    nt = NTILES
    for k in range(N_CORES):
        st = results[k]["out"].astype(np.float64).sum(axis=0)  # [OUT_COLS]
        dv = st[ACT_COLS:]
        if USE_ACT_ACCUM:
            u_all = st[0 : C * nt].sum()
            logp_s = st[C * nt : C * nt + nt].sum()
        else:
            u_all = dv[4 * nt : 5 * nt].sum()
            logp_s = dv[5 * nt : 6 * nt].sum()
        neg_a = dv[0:nt].sum()
        neg_logp = dv[nt : 2 * nt].sum()
        lsel_s = dv[2 * nt : 3 * nt].sum()
        neg_lsel = dv[3 * nt : 4 * nt].sum()
        s_all += -(u_all + logp_s - lsel_s)
        s_neg += -(neg_a + neg_logp - neg_lsel)

    s_pos = s_all - s_neg
    loss = ((neg / tot) * s_pos + (pos / tot) * s_neg) / (tot * C)
    return np.float32(loss)


_NC_CACHE = None


def kernel(predict: np.ndarray, target: np.ndarray) -> np.ndarray:
    global _NC_CACHE
    if _NC_CACHE is None:
        _NC_CACHE = build_kernel()
    nc = _NC_CACHE

    target = np.ascontiguousarray(target, dtype=np.int32)
    in_maps = make_in_maps(predict, target)
    res = run_bass_kernel_spmd(nc, in_maps, list(range(N_CORES)))
    return combine_results(res.results, target)
